# revision 7
# baseline (speedup 1.0000x reference)
"""LookAheadMask kernel for Trainium2.

out[b, r, c] = 1.0 if c > r else x[b, r, c], for x of shape (8, 4096, 4096) f32.

Sharding: batch dim across 8 NeuronCores (data parallel, no communication).

Per-core plan (matrix is S x S, S=4096, row-blocks of P=128), raw bass
(explicit engines + semaphores; the Tile drain would exceed walrus's
sync-wait-slot limit with this many independent DMAs):

  - strictly-lower region (cols < block start): 31 direct DRAM->DRAM copies
  - strictly-upper region (cols >= block end): 31 DMAs from an SBUF ones
    tile (no HBM read for that half)
  - the 32 diagonal 128x128 blocks: one 3D-strided gather DMA into SBUF
    [128, 32*128], one gpsimd affine_select (keep x where
    row >= col-within-block, else 1.0), one scatter back.

A single HWDGE ring executes queued DMAs one at a time (profiled: slice
durations sum to the whole span), so the 62 bulk DMAs are split round-robin
across three descriptor paths that run concurrently: SP ring (sync), ACT
ring (scalar), and SWDGE (gpsimd). Row-block i's copy (i*64KB) and ones
((31-i)*64KB) pair to ~2MB, so assigning pairs round-robin balances bytes.

HBM traffic/core: ~33 MiB read + 64 MiB write vs 128 MiB naive.
"""

import numpy as np

from concourse import bass, mybir
from concourse.bass_utils import run_bass_kernel_spmd

S = 4096
P = 128
NB = S // P  # 32
N_CORES = 8

_cached_nc = None


def _build():
    global _cached_nc
    if _cached_nc is not None:
        return _cached_nc

    nc = bass.Bass()
    x = nc.dram_tensor("x", [S, S], mybir.dt.float32, kind="ExternalInput")
    out = nc.dram_tensor("out", [S, S], mybir.dt.float32, kind="ExternalOutput")

    # Diagonal-block view: [row-in-block(128), block(32), col-in-block(128)],
    # block b starts at element offset b*(P*S + P). Strides in elements.
    diag_pairs = [[S, P], [P * S + P, NB], [1, P]]
    # Gather window: W cols per diag block ending at its right edge, so
    # descriptors are W*4 bytes instead of 512 (the 512B-descriptor gather
    # profiled at 152us for 2MB). Blocks 1..31 in one DMA; block 0's window
    # would start before the tensor, so it gets its own 128-col load.
    W = 256

    # All 62 bulk DMAs on the SP HWDGE ring (dsem); the ACT ring carries
    # only the wide-window diag gather + scatter, far off the critical path

    def bulk(eng, blocks, ones):
        """Emit copy then ones DMAs for the given row-blocks on one engine."""
        for i in blocks:
            r0 = i * P
            if i > 0:
                eng.dma_start(
                    out=out[r0 : r0 + P, 0:r0], in_=x[r0 : r0 + P, 0:r0]
                ).then_inc(dsem, 16)
        eng.wait_ge(msem, 1)
        for i in blocks:
            r0 = i * P
            if i < NB - 1:
                w = S - r0 - P
                eng.dma_start(
                    out=out[r0 : r0 + P, r0 + P : S], in_=ones[:, :w]
                ).then_inc(dsem, 16)

    with (
        nc.Block() as block,
        nc.semaphore("dsem") as dsem,  # bulk DMA completions (HWDGE rings)
        nc.semaphore("gsem") as gsem,  # diag gather done
        nc.semaphore("ssem") as ssem,  # diag scatter done
        nc.semaphore("msem") as msem,  # ones memset done
        nc.semaphore("asem") as asem,  # affine_select done
        nc.sbuf_tensor("ones", [P, S], mybir.dt.float32) as ones,
        nc.sbuf_tensor("diag_in2", [P, NB * W], mybir.dt.float32) as diag_in2,
        nc.sbuf_tensor("diag_out", [P, S], mybir.dt.float32) as diag_out,
    ):

        @block.vector
        def _(vector: bass.BassVectorEngine):
            vector.memset(ones[:, :], 1.0).then_inc(msem, 1)

        @block.scalar
        def _(scalar: bass.BassEngine):
            scalar.dma_start(
                out=bass.AP(diag_in2, W, [[NB * W, P], [W, NB - 1], [1, W]]),
                in_=bass.AP(x, (P * S + P) + P - W, [[S, P], [P * S + P, NB - 1], [1, W]]),
            ).then_inc(gsem, 16)
            scalar.dma_start(
                out=bass.AP(diag_in2, W - P, [[NB * W, P], [1, P]]),
                in_=x[0:P, 0:P],
            ).then_inc(gsem, 16)
            scalar.wait_ge(asem, 1)
            scalar.dma_start(
                out=bass.AP(out, 0, diag_pairs), in_=diag_out[:, :]
            ).then_inc(ssem, 16)

        @block.gpsimd
        def _(gpsimd: bass.BassGpSimd):
            gpsimd.wait_ge(gsem, 32)
            # iota[p, c] = p - (c % 128); keep x where >= 0 (at/below diag).
            # Input reads the last 128 cols of each W-wide gathered block.
            gpsimd.affine_select(
                out=diag_out[:, :],
                in_=bass.AP(diag_in2, W - P, [[NB * W, P], [W, NB], [1, P]]),
                pattern=[[0, NB], [-1, P]],
                base=0,
                channel_multiplier=1,
                compare_op=mybir.AluOpType.is_ge,
                fill=1.0,
            ).then_inc(asem, 1)

        @block.sync
        def _(sync: bass.BassEngine):
            bulk(sync, range(NB), ones)
            sync.wait_ge(dsem, 16 * 62)
            sync.wait_ge(ssem, 16)

    _cached_nc = nc
    return nc


def _run(x_full: np.ndarray, trace: bool = False):
    nc = _build()
    x_full = np.asarray(x_full, dtype=np.float32)
    in_maps = [{"x": x_full[i]} for i in range(N_CORES)]
    res = run_bass_kernel_spmd(nc, in_maps, list(range(N_CORES)), trace=trace)
    out = np.stack([res.results[i]["out"] for i in range(N_CORES)], axis=0)
    return out, res


def kernel(x: np.ndarray) -> np.ndarray:
    out, _ = _run(x, trace=False)
    return out


# revision 8
# speedup vs baseline: 1.2245x; 1.2245x over previous
"""LookAheadMask kernel for Trainium2.

out[b, r, c] = 1.0 if c > r else x[b, r, c], for x of shape (8, 4096, 4096) f32.

Sharding: batch dim across 8 NeuronCores (data parallel, no communication).

Per-core plan (matrix is S x S, S=4096, row-blocks of P=128), raw bass
(explicit engines + semaphores; the Tile drain would exceed walrus's
sync-wait-slot limit with this many independent DMAs):

  - strictly-lower region (cols < block start): 31 direct DRAM->DRAM copies
  - strictly-upper region (cols >= block end): 31 DMAs from an SBUF ones
    tile (no HBM read for that half)
  - the 32 diagonal 128x128 blocks: one 3D-strided gather DMA into SBUF
    [128, 32*128], one gpsimd affine_select (keep x where
    row >= col-within-block, else 1.0), one scatter back.

A single HWDGE ring executes queued DMAs one at a time (profiled: slice
durations sum to the whole span), so the 62 bulk DMAs are split round-robin
across three descriptor paths that run concurrently: SP ring (sync), ACT
ring (scalar), and SWDGE (gpsimd). Row-block i's copy (i*64KB) and ones
((31-i)*64KB) pair to ~2MB, so assigning pairs round-robin balances bytes.

HBM traffic/core: ~33 MiB read + 64 MiB write vs 128 MiB naive.
"""

import numpy as np

from concourse import bass, mybir
from concourse.bass_utils import run_bass_kernel_spmd

S = 4096
P = 128
NB = S // P  # 32
N_CORES = 8

_cached_nc = None


def _build():
    global _cached_nc
    if _cached_nc is not None:
        return _cached_nc

    nc = bass.Bass()
    x = nc.dram_tensor("x", [S, S], mybir.dt.float32, kind="ExternalInput")
    out = nc.dram_tensor("out", [S, S], mybir.dt.float32, kind="ExternalOutput")

    # Diagonal-block view: [row-in-block(128), block(32), col-in-block(128)],
    # block b starts at element offset b*(P*S + P). Strides in elements.
    diag_pairs = [[S, P], [P * S + P, NB], [1, P]]
    # Gather window: W cols per diag block ending at its right edge, so
    # descriptors are W*4 bytes instead of 512 (the 512B-descriptor gather
    # profiled at 152us for 2MB). Blocks 1..31 in one DMA; block 0's window
    # would start before the tensor, so it gets its own 128-col load.
    W = 256

    # 62 bulk DMAs all HWDGE (dsem): 47 on the SP ring, 15 on the ACT ring
    # (issued after the ACT ring's cheap wide-window diag gather)

    def bulk(eng, blocks, ones):
        """Emit copy then ones DMAs for the given row-blocks on one engine."""
        for i in blocks:
            r0 = i * P
            if i > 0:
                eng.dma_start(
                    out=out[r0 : r0 + P, 0:r0], in_=x[r0 : r0 + P, 0:r0]
                ).then_inc(dsem, 16)
        eng.wait_ge(msem, 1)
        for i in blocks:
            r0 = i * P
            if i < NB - 1:
                w = S - r0 - P
                eng.dma_start(
                    out=out[r0 : r0 + P, r0 + P : S], in_=ones[:, :w]
                ).then_inc(dsem, 16)

    with (
        nc.Block() as block,
        nc.semaphore("dsem") as dsem,  # bulk DMA completions (HWDGE rings)
        nc.semaphore("gsem") as gsem,  # diag gather done
        nc.semaphore("ssem") as ssem,  # diag scatter done
        nc.semaphore("msem") as msem,  # ones memset done
        nc.semaphore("asem") as asem,  # affine_select done
        nc.sbuf_tensor("ones", [P, S], mybir.dt.float32) as ones,
        nc.sbuf_tensor("diag_in2", [P, NB * W], mybir.dt.float32) as diag_in2,
        nc.sbuf_tensor("diag_out", [P, S], mybir.dt.float32) as diag_out,
    ):

        @block.vector
        def _(vector: bass.BassVectorEngine):
            vector.memset(ones[:, :], 1.0).then_inc(msem, 1)

        @block.scalar
        def _(scalar: bass.BassEngine):
            scalar.dma_start(
                out=bass.AP(diag_in2, W, [[NB * W, P], [W, NB - 1], [1, W]]),
                in_=bass.AP(x, (P * S + P) + P - W, [[S, P], [P * S + P, NB - 1], [1, W]]),
            ).then_inc(gsem, 16)
            scalar.dma_start(
                out=bass.AP(diag_in2, W - P, [[NB * W, P], [1, P]]),
                in_=x[0:P, 0:P],
            ).then_inc(gsem, 16)
            bulk(scalar, range(3, NB, 4), ones)
            scalar.wait_ge(asem, 1)
            scalar.dma_start(
                out=bass.AP(out, 0, diag_pairs), in_=diag_out[:, :]
            ).then_inc(ssem, 16)

        @block.gpsimd
        def _(gpsimd: bass.BassGpSimd):
            gpsimd.wait_ge(gsem, 32)
            # iota[p, c] = p - (c % 128); keep x where >= 0 (at/below diag).
            # Input reads the last 128 cols of each W-wide gathered block.
            gpsimd.affine_select(
                out=diag_out[:, :],
                in_=bass.AP(diag_in2, W - P, [[NB * W, P], [W, NB], [1, P]]),
                pattern=[[0, NB], [-1, P]],
                base=0,
                channel_multiplier=1,
                compare_op=mybir.AluOpType.is_ge,
                fill=1.0,
            ).then_inc(asem, 1)

        @block.sync
        def _(sync: bass.BassEngine):
            bulk(sync, [i for i in range(NB) if i % 4 != 3], ones)
            sync.wait_ge(dsem, 16 * 62)
            sync.wait_ge(ssem, 16)

    _cached_nc = nc
    return nc


def _run(x_full: np.ndarray, trace: bool = False):
    nc = _build()
    x_full = np.asarray(x_full, dtype=np.float32)
    in_maps = [{"x": x_full[i]} for i in range(N_CORES)]
    res = run_bass_kernel_spmd(nc, in_maps, list(range(N_CORES)), trace=trace)
    out = np.stack([res.results[i]["out"] for i in range(N_CORES)], axis=0)
    return out, res


def kernel(x: np.ndarray) -> np.ndarray:
    out, _ = _run(x, trace=False)
    return out
